# revision 26
# baseline (speedup 1.0000x reference)
"""Trainium2 Bass kernel for nn_Disentangler (gnn_message_passing).

Reference computation per timestamp t (T=16):
  xn   = LayerNorm_E(x[t])                 [16384, 128] -> first 8192 rows used
  tee  = segment_sum(xn[:8192] by node_idx[t])      [50000, 128]
  pool = blockmean_4(tee)                           [50000, 32]
  agg  = mean over basket slots of pool[stacked]    [64, 32]
  out  = LayerNorm_2048(agg.reshape(1, 2048))

Algebraic reformulation (all FP math on x happens on device):
  For token i with node n_i, A[i, j] = (# occurrences of n_i among basket j's
  782 slots) — an integer count matrix derived purely from the two index
  tensors (host-side index preprocessing).  With per-token LN1 stats
  (m_i, r_i = rsqrt(var_i+eps)), q_i[c] = sum_{e in block c} x[i,e]*g1[e],
  sc[c] = sum_block g1, bb[c] = mean_block b1:

    agg[j, c] = (1/782) * [ sum_i A[i,j]*u_i[c]        (u = q * r/4)
                            - sc[c] * sum_i A[i,j]*w_i  (w = m * r/4)
                            + bb[c] * sum_i A[i,j] ]

  i.e. one token-contraction matmul  A^T @ [u | 1 | w]  per timestamp.
  Tokens whose node appears in no basket have A == 0 and are dropped
  host-side (packed token list, ~5.2k of 8192; padded to NT=5632).

Sharding: data-parallel over T (2 timestamps per core, 8 cores).

Device pipeline per timestamp:
  1. xT [E=128, NT] bf16 <- HWDGE dma_start_transpose of packed x rows,
     in 4 pieces so stats matmuls pipeline with the load
  2. sq chunks = xT*xT (DVE, per 512-token chunk)
  3. stats: per 512-chunk, [0|ssq] selector matmul lands on PSUM rows 32-33
     (base 32), then the 33-col [Wg|1] matmul at base 0 overwrites row 32
     with sum_x (program-order WAW) -> one [34, 512] PSUM tile per chunk,
     evacuated alternately by ACT/DVE to stats_e [34, NT] bf16
  4. 44 PE transposes [34,128]->[128,34] -> token-major stats
  5. tiny token-major DVE/ACT ops -> r4, u, w  (rhs2 = [u | 1 | w] bf16)
  6. 44 accumulating matmuls psC[64,34] = A-chunk^T @ rhs2-chunk
  7. agg finalize + LayerNorm(2048); global sums + broadcast via three tiny
     matmuls; output [64, 32] f32 -> HBM.
"""

import os
import sys

import ml_dtypes
import numpy as np

# ---------------------------------------------------------------- constants
T = 16
TOK = 16384
E = 128
N_NODE = 8192
NUM_NODES = 50000
COMP_LEN = 64   # J baskets
MAX_LEN = 782
COMP_DIM = 32   # C
EPS = 1e-5

N_CORES = 8
T_LOC = T // N_CORES   # 2 timestamps per core

NT = 5632              # packed tokens (kept ~5186 +- 44; 10 sigma headroom)
CH = NT // 128         # 44 token chunks
NK = NT // 512         # 11 stats matmul chunks
NSTAT = 34             # stats rows: [q(32) | sum_x | sum_x2]
NSTATP = 48            # stats rows padded to x16 for the xbar DMA transpose
XPIECES = (1536, 1536, 1536, 1024)   # xT load pieces (multiples of 512)
R4S = 0.25 / MAX_LEN   # folded r/4 * 1/max_len scale

_PROGRAM = None
LAST_RESULTS = None    # BassKernelResults of the last run (for test harness)

BF16 = ml_dtypes.bfloat16


def _build_program():
    import concourse.bacc as bacc
    import concourse.bass as bass
    import concourse.mybir as mybir
    import concourse.tile as tile
    from concourse import masks

    f32 = mybir.dt.float32
    bf16 = mybir.dt.bfloat16

    nc = bacc.Bacc("TRN2", target_bir_lowering=False, debug=False,
                   num_devices=N_CORES)

    xb_d = nc.dram_tensor("xb", [T_LOC, NT, E], bf16, kind="ExternalInput")
    am_d = nc.dram_tensor("am", [T_LOC, 128, CH, COMP_LEN], bf16,
                          kind="ExternalInput")
    wstat_d = nc.dram_tensor("wstat", [E, NSTAT], bf16, kind="ExternalInput")
    sc_d = nc.dram_tensor("sc782", [COMP_LEN, COMP_DIM], f32,
                          kind="ExternalInput")
    bb_d = nc.dram_tensor("bb782", [COMP_LEN, COMP_DIM], f32,
                          kind="ExternalInput")
    g2_d = nc.dram_tensor("g2", [COMP_LEN, COMP_DIM], f32, kind="ExternalInput")
    b2_d = nc.dram_tensor("b2", [COMP_LEN, COMP_DIM], f32, kind="ExternalInput")
    out_d = nc.dram_tensor("out", [T_LOC, COMP_LEN, COMP_DIM], f32,
                           kind="ExternalOutput")

    with tile.TileContext(nc) as tc:
        with (
            tc.tile_pool(name="const", bufs=1) as cp,
            tc.tile_pool(name="main", bufs=2) as pool,
            tc.tile_pool(name="small", bufs=2) as sp,
            tc.tile_pool(name="ps", bufs=4, space=bass.MemorySpace.PSUM) as psp,
            tc.tile_pool(name="psc", bufs=2, space=bass.MemorySpace.PSUM) as pscp,
            tc.tile_pool(name="psde", bufs=1, space=bass.MemorySpace.PSUM) as psdep,
        ):
            # ---- constants
            wstat = cp.tile([E, NSTAT], bf16)
            nc.sync.dma_start(wstat[:], wstat_d.ap())
            sc = cp.tile([COMP_LEN, COMP_DIM], f32)
            nc.sync.dma_start(sc[:], sc_d.ap())
            bb = cp.tile([COMP_LEN, COMP_DIM], f32)
            nc.sync.dma_start(bb[:], bb_d.ap())
            g2 = cp.tile([COMP_LEN, COMP_DIM], f32)
            nc.sync.dma_start(g2[:], g2_d.ap())
            b2 = cp.tile([COMP_LEN, COMP_DIM], f32)
            nc.sync.dma_start(b2[:], b2_d.ap())
            ones64 = cp.tile([COMP_LEN, 1], f32)
            nc.gpsimd.memset(ones64[:], 1.0)
            ones6464 = cp.tile([COMP_LEN, COMP_LEN], f32)
            nc.gpsimd.memset(ones6464[:], 1.0)
            # sel2 carries the 1/2048 LN2-mean scale
            sel2 = cp.tile([COMP_LEN, 2], f32)
            nc.gpsimd.memset(sel2[:], 0.0)
            nc.gpsimd.memset(sel2[0:COMP_DIM, 0:1], 1.0 / 2048.0)
            nc.gpsimd.memset(sel2[COMP_DIM:COMP_LEN, 1:2], 1.0 / 2048.0)
            epsb = cp.tile([128, 1], f32)
            nc.gpsimd.memset(epsb[:], EPS)
            # [zero | ones] selector: lhsT for the sum_x2 row
            ssqsel = cp.tile([E, 2], bf16)
            nc.gpsimd.memset(ssqsel[:, 0:1], 0.0)
            nc.gpsimd.memset(ssqsel[:, 1:2], 1.0)
            warm = cp.tile([E, 512], bf16)
            nc.gpsimd.memset(warm[:], 0.5)

            # PE p-state warmup burst (~4 us) while the first x piece loads
            psw = psp.tile([NSTAT, 512], f32, tag="psA")
            for _ in range(18):
                nc.tensor.matmul(psw[0:33, :], wstat[:, 0:33], warm[:],
                                 start=True, stop=True)

            for t in range(T_LOC):
                # ---- 1. transposed load of packed x rows, in pieces
                xT = pool.tile([128, NT], bf16, tag="xT")
                off = 0
                for plen in XPIECES:
                    nc.sync.dma_start_transpose(
                        xT[:, off:off + plen],
                        xb_d.ap()[t, off:off + plen, :])
                    off += plen

                # ---- A matrix (host-prepared counts, chunk layout)
                a_sb = pool.tile([128, CH, COMP_LEN], bf16, tag="A")
                nc.sync.dma_start(a_sb[:], am_d.ap()[t])

                # ---- 2+3. per-chunk square + stats matmuls
                sqT = pool.tile([128, NT], bf16, tag="sqT")
                stats_e = pool.tile([NSTATP, NT], bf16, tag="stats_e")
                # zero the pad rows (start partition must be 32-aligned;
                # rows 32-33 are rewritten by the stats evacs afterwards)
                nc.gpsimd.memset(stats_e[32:NSTATP, :], 0.0)
                for k in range(NK):
                    ksl = slice(k * 512, (k + 1) * 512)
                    if k % 3 == 2:
                        nc.scalar.square(sqT[:, ksl], xT[:, ksl])
                    else:
                        nc.vector.tensor_mul(sqT[:, ksl], xT[:, ksl], xT[:, ksl])
                    ps = psp.tile([NSTAT, 512], f32, tag="psA")
                    nc.tensor.matmul(ps[32:34, :], ssqsel[:], sqT[:, ksl],
                                     start=True, stop=True)
                    nc.tensor.matmul(ps[0:33, :], wstat[:, 0:33], xT[:, ksl],
                                     start=True, stop=True)
                    if k % 2 == 0:
                        nc.scalar.copy(stats_e[0:NSTAT, ksl], ps[:])
                    else:
                        nc.vector.tensor_copy(stats_e[0:NSTAT, ksl], ps[:])

                # ---- 4. stats to token-major: ONE xbar DMA transpose
                # [48, NT] -> [128, CH, 48]: out[p, g, s] = stats_e[s, g*128+p]
                stats_tok = pool.tile([128, CH, NSTATP], bf16, tag="stats_tok")
                nc.sync.dma_start_transpose(stats_tok[:], stats_e[:])

                # ---- 5. per-token scalars (all [128, CH], tiny)
                m_f = sp.tile([128, CH], f32, tag="m")
                nc.vector.tensor_scalar_mul(m_f[:], stats_tok[:, :, 32], 1.0 / E)
                v_f = sp.tile([128, CH], f32, tag="v")
                nc.vector.tensor_scalar_mul(v_f[:], stats_tok[:, :, 33], 1.0 / E)
                m2_f = sp.tile([128, CH], f32, tag="m2")
                nc.vector.tensor_mul(m2_f[:], m_f[:], m_f[:])
                nc.vector.tensor_sub(v_f[:], v_f[:], m2_f[:])
                sd_f = sp.tile([128, CH], f32, tag="sd")
                nc.scalar.activation(sd_f[:], v_f[:],
                                     mybir.ActivationFunctionType.Sqrt,
                                     bias=epsb[:])
                ri_f = sp.tile([128, CH], f32, tag="ri")
                nc.vector.reciprocal(ri_f[:], sd_f[:])
                r4_b = sp.tile([128, CH], bf16, tag="r4")
                nc.vector.tensor_scalar_mul(r4_b[:], ri_f[:], R4S)
                w_f = sp.tile([128, CH], f32, tag="w")
                nc.vector.tensor_mul(w_f[:], m_f[:], ri_f[:])

                rhs2 = pool.tile([128, CH, NSTAT], bf16, tag="rhs2")
                nc.vector.tensor_mul(
                    rhs2[:, :, 0:COMP_DIM], stats_tok[:, :, 0:COMP_DIM],
                    r4_b[:].unsqueeze(2).broadcast_to([128, CH, COMP_DIM]))
                nc.gpsimd.memset(rhs2[:, :, 32:33], 1.0)
                nc.vector.tensor_scalar_mul(rhs2[:, :, 33], w_f[:], R4S)

                # ---- 6. token contraction
                psc = pscp.tile([COMP_LEN, NSTAT], f32, tag="psC")
                for g in range(CH):
                    nc.tensor.matmul(psc[:], a_sb[:, g, :], rhs2[:, g, :],
                                     start=(g == 0), stop=(g == CH - 1))

                # ---- 7. agg finalize ([64, 32] fp32, tiny)
                # cat cols 0-31 already carry q*r/(4*782); kappa unscaled.
                cat = sp.tile([COMP_LEN, NSTAT], f32, tag="cat")
                nc.scalar.copy(cat[:], psc[:])
                t1 = sp.tile([COMP_LEN, COMP_DIM], f32, tag="t1")
                nc.vector.tensor_mul(
                    t1[:], cat[:, 33:34].broadcast_to([COMP_LEN, COMP_DIM]),
                    sc[:])
                t2 = sp.tile([COMP_LEN, COMP_DIM], f32, tag="t2")
                nc.vector.tensor_mul(
                    t2[:], cat[:, 32:33].broadcast_to([COMP_LEN, COMP_DIM]),
                    bb[:])
                nc.vector.tensor_sub(t2[:], t2[:], t1[:])
                cat2 = sp.tile([COMP_LEN, 2 * COMP_DIM], f32, tag="cat2")
                nc.vector.tensor_add(cat2[:, 0:COMP_DIM], cat[:, 0:COMP_DIM],
                                     t2[:])
                nc.vector.tensor_mul(cat2[:, COMP_DIM:2 * COMP_DIM],
                                     cat2[:, 0:COMP_DIM], cat2[:, 0:COMP_DIM])

                # ---- LN2 global sums + broadcast via PE (sel2 carries 1/2048)
                psd = psdep.tile([COMP_LEN, 1], f32, tag="psDE")
                nc.tensor.matmul(psd[:], cat2[:], ones64[:],
                                 start=True, stop=True)
                sD = sp.tile([COMP_LEN, 1], f32, tag="sD")
                nc.vector.tensor_copy(sD[:], psd[:])
                sDm = sp.tile([COMP_LEN, 2], f32, tag="sDm")
                nc.vector.tensor_mul(
                    sDm[:], sD[:].broadcast_to([COMP_LEN, 2]), sel2[:])
                psf = psdep.tile([COMP_LEN, 2], f32, tag="psDE")
                nc.tensor.matmul(psf[:], ones6464[:], sDm[:],
                                 start=True, stop=True)
                bS = sp.tile([COMP_LEN, 2], f32, tag="bS")
                nc.vector.tensor_copy(bS[:], psf[:])

                mu = bS[:, 0:1]
                mu2 = sp.tile([COMP_LEN, 1], f32, tag="mu2")
                nc.vector.tensor_mul(mu2[:], bS[:, 0:1], bS[:, 0:1])
                ex2 = sp.tile([COMP_LEN, 1], f32, tag="ex2")
                nc.vector.tensor_sub(ex2[:], bS[:, 1:2], mu2[:])
                sd2 = sp.tile([COMP_LEN, 1], f32, tag="sd2")
                nc.scalar.activation(sd2[:], ex2[:],
                                     mybir.ActivationFunctionType.Sqrt,
                                     bias=epsb[0:COMP_LEN, :])
                rr = sp.tile([COMP_LEN, 1], f32, tag="rr")
                nc.vector.reciprocal(rr[:], sd2[:])

                obuf = sp.tile([COMP_LEN, COMP_DIM], f32, tag="obuf")
                nc.vector.tensor_scalar(obuf[:], cat2[:, 0:COMP_DIM],
                                        mu, rr[:],
                                        mybir.AluOpType.subtract,
                                        mybir.AluOpType.mult)
                nc.vector.tensor_mul(obuf[:], obuf[:], g2[:])
                nc.vector.tensor_add(obuf[:], obuf[:], b2[:])

                nc.sync.dma_start(out_d.ap()[t], obuf[:])

    nc.compile()
    return nc


def _get_program():
    global _PROGRAM
    if _PROGRAM is None:
        _PROGRAM = _build_program()
    return _PROGRAM


def _prepare_inputs(x, ln1_g, ln1_b, ln2_g, ln2_b, node_idx, stacked_indices):
    """Host-side index preprocessing + weight prep. Returns list of in_maps."""
    node_idx = np.asarray(node_idx).astype(np.int64)
    stacked = np.asarray(stacked_indices).astype(np.int64)
    x = np.asarray(x, dtype=np.float32)
    ln1_g = np.asarray(ln1_g, dtype=np.float32)
    ln1_b = np.asarray(ln1_b, dtype=np.float32)
    ln2_g = np.asarray(ln2_g, dtype=np.float32)
    ln2_b = np.asarray(ln2_b, dtype=np.float32)

    # histogram bt[n, j] = count of node n in basket j  (index preprocessing)
    bt = np.zeros((NUM_NODES, COMP_LEN), dtype=np.float32)
    j_ids = np.broadcast_to(np.arange(COMP_LEN)[:, None], stacked.shape)
    np.add.at(bt, (stacked.ravel(), j_ids.ravel()), 1.0)
    node_used = bt.any(axis=1)

    # weight prep
    wstat = np.zeros((E, NSTAT), dtype=np.float32)
    wstat[np.arange(E), np.arange(E) // 4] = ln1_g
    wstat[:, 32] = 1.0
    wstat_bf = wstat.astype(BF16)
    scv = ln1_g.reshape(COMP_DIM, 4).sum(1)
    bbv = ln1_b.reshape(COMP_DIM, 4).mean(1)
    # sc is used against lambda which already carries 1/max_len (via R4S)
    sc782 = np.broadcast_to(scv, (COMP_LEN, COMP_DIM)).copy()
    bb782 = np.broadcast_to(bbv / MAX_LEN, (COMP_LEN, COMP_DIM)).copy()
    g2 = np.ascontiguousarray(ln2_g.reshape(COMP_LEN, COMP_DIM))
    b2 = np.ascontiguousarray(ln2_b.reshape(COMP_LEN, COMP_DIM))

    in_maps = []
    for core in range(N_CORES):
        ts = list(range(core * T_LOC, (core + 1) * T_LOC))
        am = np.zeros((T_LOC, 128, CH, COMP_LEN), dtype=BF16)
        xb = np.empty((T_LOC, NT, E), dtype=BF16)
        for ti, tg in enumerate(ts):
            nt_ids = node_idx[tg, :N_NODE]
            kept = np.flatnonzero(node_used[nt_ids])
            if len(kept) > NT:
                print(f"WARNING: kept token overflow {len(kept)} > {NT}",
                      file=sys.stderr)
                kept = kept[:NT]
            nk = len(kept)
            sel = np.zeros(NT, dtype=np.int64)
            sel[:nk] = kept
            xb[ti] = x[tg, sel, :].astype(BF16)
            a_full = bt[nt_ids[sel], :]
            a_full[nk:, :] = 0.0
            am[ti] = a_full.reshape(CH, 128, COMP_LEN).transpose(1, 0, 2)
        in_maps.append({
            "xb": xb,
            "am": am,
            "wstat": wstat_bf,
            "sc782": sc782.astype(np.float32),
            "bb782": bb782.astype(np.float32),
            "g2": g2.astype(np.float32),
            "b2": b2.astype(np.float32),
        })
    return in_maps


def kernel(x, ln1_g, ln1_b, ln2_g, ln2_b, node_idx, stacked_indices,
           n_node=N_NODE, num_nodes=NUM_NODES):
    global LAST_RESULTS
    from concourse.bass_utils import run_bass_kernel_spmd

    nc = _get_program()
    in_maps = _prepare_inputs(x, ln1_g, ln1_b, ln2_g, ln2_b, node_idx,
                              stacked_indices)

    if os.environ.get("KERNEL_SIM"):
        outs = _run_sim(nc, in_maps)
    else:
        res = run_bass_kernel_spmd(
            nc, in_maps, core_ids=list(range(N_CORES)),
            trace=bool(os.environ.get("KERNEL_TRACE")),
        )
        LAST_RESULTS = res
        outs = [r["out"] for r in res.results]

    full = np.concatenate(outs, axis=0)           # [16, 64, 32]
    return full.reshape(T, 1, COMP_LEN * COMP_DIM).astype(np.float32)


def _run_sim(nc, in_maps):
    """CoreSim path (KERNEL_SIM=1): simulate cores serially."""
    from concourse.bass_interp import CoreSim
    outs = []
    ncores = int(os.environ.get("KERNEL_SIM_CORES", "1"))
    for core, im in enumerate(in_maps[:ncores]):
        sim = CoreSim(nc, trace=False)
        for k, v in im.items():
            sim.tensor(k)[:] = v
        sim.simulate(check_with_hw=False)
        outs.append(np.array(sim.tensor("out")))
    for core in range(ncores, len(in_maps)):
        outs.append(np.zeros((T_LOC, COMP_LEN, COMP_DIM), np.float32))
    return outs


# revision 33
# speedup vs baseline: 1.0058x; 1.0058x over previous
"""Trainium2 Bass kernel for nn_Disentangler (gnn_message_passing).

Reference computation per timestamp t (T=16):
  xn   = LayerNorm_E(x[t])                 [16384, 128] -> first 8192 rows used
  tee  = segment_sum(xn[:8192] by node_idx[t])      [50000, 128]
  pool = blockmean_4(tee)                           [50000, 32]
  agg  = mean over basket slots of pool[stacked]    [64, 32]
  out  = LayerNorm_2048(agg.reshape(1, 2048))

Algebraic reformulation (all FP math on x happens on device):
  For token i with node n_i, A[i, j] = (# occurrences of n_i among basket j's
  782 slots) — an integer count matrix derived purely from the two index
  tensors (host-side index preprocessing).  With per-token LN1 stats
  (m_i, r_i = rsqrt(var_i+eps)), q_i[c] = sum_{e in block c} x[i,e]*g1[e],
  sc[c] = sum_block g1, bb[c] = mean_block b1:

    agg[j, c] = (1/782) * [ sum_i A[i,j]*u_i[c]        (u = q * r/4)
                            - sc[c] * sum_i A[i,j]*w_i  (w = m * r/4)
                            + bb[c] * sum_i A[i,j] ]

  i.e. one token-contraction matmul  A^T @ [u | 1 | w]  per timestamp.
  Tokens whose node appears in no basket have A == 0 and are dropped
  host-side (packed token list, ~5.2k of 8192; padded to NT=5632).

Sharding: data-parallel over T (2 timestamps per core, 8 cores).

Device pipeline per timestamp:
  1. xT [E=128, NT] bf16 <- HWDGE dma_start_transpose of packed x rows,
     in 4 pieces so stats matmuls pipeline with the load
  2. sq chunks = xT*xT (DVE, per 512-token chunk)
  3. stats: per 512-chunk, [0|ssq] selector matmul lands on PSUM rows 32-33
     (base 32), then the 33-col [Wg|1] matmul at base 0 overwrites row 32
     with sum_x (program-order WAW) -> one [34, 512] PSUM tile per chunk,
     evacuated alternately by ACT/DVE to stats_e [34, NT] bf16
  4. 44 PE transposes [34,128]->[128,34] -> token-major stats
  5. tiny token-major DVE/ACT ops -> r4, u, w  (rhs2 = [u | 1 | w] bf16)
  6. 44 accumulating matmuls psC[64,34] = A-chunk^T @ rhs2-chunk
  7. agg finalize + LayerNorm(2048); global sums + broadcast via three tiny
     matmuls; output [64, 32] f32 -> HBM.
"""

import os
import sys

import ml_dtypes
import numpy as np

# ---------------------------------------------------------------- constants
T = 16
TOK = 16384
E = 128
N_NODE = 8192
NUM_NODES = 50000
COMP_LEN = 64   # J baskets
MAX_LEN = 782
COMP_DIM = 32   # C
EPS = 1e-5

N_CORES = 8
T_LOC = T // N_CORES   # 2 timestamps per core

NT = 5632              # packed tokens (kept ~5186 +- 44; 10 sigma headroom)
CH = NT // 128         # 44 token chunks
NK = NT // 512         # 11 stats matmul chunks
NSTAT = 34             # stats rows: [q(32) | sum_x | sum_x2]
NSTATP = 48            # stats rows padded to x16 for the xbar DMA transpose
XPIECES = (1536, 1536, 1536, 1024)   # xT load pieces (multiples of 512)
R4S = 0.25 / MAX_LEN   # folded r/4 * 1/max_len scale

_PROGRAM = None
LAST_RESULTS = None    # BassKernelResults of the last run (for test harness)

BF16 = ml_dtypes.bfloat16


def _build_program():
    import concourse.bacc as bacc
    import concourse.bass as bass
    import concourse.mybir as mybir
    import concourse.tile as tile
    from concourse import masks

    f32 = mybir.dt.float32
    bf16 = mybir.dt.bfloat16

    nc = bacc.Bacc("TRN2", target_bir_lowering=False, debug=False,
                   num_devices=N_CORES)

    xb_d = nc.dram_tensor("xb", [T_LOC, NT, E], bf16, kind="ExternalInput")
    am_d = nc.dram_tensor("am", [T_LOC, 128, CH, COMP_LEN], bf16,
                          kind="ExternalInput")
    wstat_d = nc.dram_tensor("wstat", [E, NSTAT], bf16, kind="ExternalInput")
    # packed [sc | bb | g2 | b2] as one [64, 128] f32 input
    cst_d = nc.dram_tensor("cst4", [COMP_LEN, 4 * COMP_DIM], f32,
                           kind="ExternalInput")
    out_d = nc.dram_tensor("out", [T_LOC, COMP_LEN, COMP_DIM], f32,
                           kind="ExternalOutput")

    with tile.TileContext(nc) as tc:
        with (
            tc.tile_pool(name="const", bufs=1) as cp,
            tc.tile_pool(name="main", bufs=2) as pool,
            tc.tile_pool(name="small", bufs=2) as sp,
            tc.tile_pool(name="ps", bufs=4, space=bass.MemorySpace.PSUM) as psp,
            tc.tile_pool(name="psc", bufs=2, space=bass.MemorySpace.PSUM) as pscp,
            tc.tile_pool(name="psde", bufs=1, space=bass.MemorySpace.PSUM) as psdep,
        ):
            # ---- constants
            wstat = cp.tile([E, NSTAT], bf16)
            nc.sync.dma_start(wstat[:], wstat_d.ap())
            cst4 = cp.tile([COMP_LEN, 4 * COMP_DIM], f32)
            nc.sync.dma_start(cst4[:], cst_d.ap())
            sc = cst4[:, 0:COMP_DIM]
            bb = cst4[:, COMP_DIM:2 * COMP_DIM]
            g2 = cst4[:, 2 * COMP_DIM:3 * COMP_DIM]
            b2 = cst4[:, 3 * COMP_DIM:4 * COMP_DIM]
            ones64 = cp.tile([COMP_LEN, 1], f32)
            nc.vector.memset(ones64[:], 1.0)
            ones6464 = cp.tile([COMP_LEN, COMP_LEN], f32)
            nc.vector.memset(ones6464[:], 1.0)
            # sel2 carries the 1/2048 LN2-mean scale
            sel2 = cp.tile([COMP_LEN, 2], f32)
            nc.vector.memset(sel2[:], 0.0)
            nc.vector.memset(sel2[0:COMP_DIM, 0:1], 1.0 / 2048.0)
            nc.vector.memset(sel2[COMP_DIM:COMP_LEN, 1:2], 1.0 / 2048.0)
            epsb = cp.tile([128, 1], f32)
            nc.vector.memset(epsb[:], EPS)
            # [zero | ones] selector: lhsT for the sum_x2 row
            ssqsel = cp.tile([E, 2], bf16)
            nc.vector.memset(ssqsel[:, 0:1], 0.0)
            nc.vector.memset(ssqsel[:, 1:2], 1.0)
            warm = cp.tile([E, 512], bf16)
            nc.vector.memset(warm[:], 0.5)

            # PE p-state warmup burst (~4 us) while the first x piece loads
            psw = psp.tile([NSTAT, 512], f32, tag="psA")
            for _ in range(18):
                nc.tensor.matmul(psw[0:33, :], wstat[:, 0:33], warm[:],
                                 start=True, stop=True)

            for t in range(T_LOC):
                # ---- 1. transposed load of packed x rows, in pieces
                xT = pool.tile([128, NT], bf16, tag="xT")
                off = 0
                for plen in XPIECES:
                    nc.sync.dma_start_transpose(
                        xT[:, off:off + plen],
                        xb_d.ap()[t, off:off + plen, :])
                    off += plen

                # ---- A matrix (host-prepared counts, chunk layout)
                a_sb = pool.tile([128, CH, COMP_LEN], bf16, tag="A")
                nc.sync.dma_start(a_sb[:], am_d.ap()[t])

                # ---- 2+3. per-chunk square + stats matmuls
                sqT = pool.tile([128, NT], bf16, tag="sqT")
                stats_e = pool.tile([NSTATP, NT], bf16, tag="stats_e")
                for k in range(NK):
                    ksl = slice(k * 512, (k + 1) * 512)
                    if k % 3 == 2:
                        nc.scalar.square(sqT[:, ksl], xT[:, ksl])
                    else:
                        nc.vector.tensor_mul(sqT[:, ksl], xT[:, ksl], xT[:, ksl])
                    ps = psp.tile([NSTAT, 512], f32, tag="psA")
                    nc.tensor.matmul(ps[32:34, :], ssqsel[:], sqT[:, ksl],
                                     start=True, stop=True)
                    nc.tensor.matmul(ps[0:33, :], wstat[:, 0:33], xT[:, ksl],
                                     start=True, stop=True)
                    if k % 2 == 0:
                        nc.scalar.copy(stats_e[0:NSTAT, ksl], ps[:])
                    else:
                        nc.vector.tensor_copy(stats_e[0:NSTAT, ksl], ps[:])

                # ---- 4-6 in two halves so the token-contraction overlaps
                # the second half's transpose + scalar chain.
                # stats_e rows 34-47 are uninitialized; they transpose into
                # stats_tok cols 34-47 which are never read.
                stats_tok = pool.tile([128, CH, NSTATP], bf16, tag="stats_tok")
                rhs2 = pool.tile([128, CH, NSTAT], bf16, tag="rhs2")
                psc = pscp.tile([COMP_LEN, NSTAT], f32, tag="psC")
                CHH = CH // 2
                for h in range(2):
                    hsl = slice(h * CHH, (h + 1) * CHH)
                    # [48, NT/2] -> [128, CH/2, 48]
                    nc.sync.dma_start_transpose(
                        stats_tok[:, hsl, :],
                        stats_e[:, h * (NT // 2):(h + 1) * (NT // 2)])

                    # per-token scalars (all [128, CH/2], tiny)
                    m_f = sp.tile([128, CHH], f32, tag="m")
                    nc.vector.tensor_scalar_mul(m_f[:], stats_tok[:, hsl, 32],
                                                1.0 / E)
                    v_f = sp.tile([128, CHH], f32, tag="v")
                    nc.vector.tensor_scalar_mul(v_f[:], stats_tok[:, hsl, 33],
                                                1.0 / E)
                    m2_f = sp.tile([128, CHH], f32, tag="m2")
                    nc.vector.tensor_mul(m2_f[:], m_f[:], m_f[:])
                    nc.vector.tensor_sub(v_f[:], v_f[:], m2_f[:])
                    sd_f = sp.tile([128, CHH], f32, tag="sd")
                    nc.scalar.activation(sd_f[:], v_f[:],
                                         mybir.ActivationFunctionType.Sqrt,
                                         bias=epsb[:])
                    ri_f = sp.tile([128, CHH], f32, tag="ri")
                    nc.vector.reciprocal(ri_f[:], sd_f[:])
                    r4_b = sp.tile([128, CHH], bf16, tag="r4")
                    nc.vector.tensor_scalar_mul(r4_b[:], ri_f[:], R4S)
                    w_f = sp.tile([128, CHH], f32, tag="w")
                    nc.vector.tensor_mul(w_f[:], m_f[:], ri_f[:])

                    nc.vector.tensor_mul(
                        rhs2[:, hsl, 0:COMP_DIM], stats_tok[:, hsl, 0:COMP_DIM],
                        r4_b[:].unsqueeze(2).broadcast_to([128, CHH, COMP_DIM]))
                    nc.vector.memset(rhs2[:, hsl, 32:33], 1.0)
                    nc.vector.tensor_scalar_mul(rhs2[:, hsl, 33], w_f[:], R4S)

                    # token contraction for this half
                    for gg in range(CHH):
                        g = h * CHH + gg
                        nc.tensor.matmul(psc[:], a_sb[:, g, :], rhs2[:, g, :],
                                         start=(g == 0), stop=(g == CH - 1))

                # ---- 7. agg finalize ([64, 32] fp32, tiny)
                # cat cols 0-31 already carry q*r/(4*782); kappa unscaled.
                cat = sp.tile([COMP_LEN, NSTAT], f32, tag="cat")
                nc.vector.tensor_copy(cat[:], psc[:])
                t1 = sp.tile([COMP_LEN, COMP_DIM], f32, tag="t1")
                nc.vector.tensor_mul(
                    t1[:], cat[:, 33:34].broadcast_to([COMP_LEN, COMP_DIM]),
                    sc)
                t2 = sp.tile([COMP_LEN, COMP_DIM], f32, tag="t2")
                nc.vector.tensor_mul(
                    t2[:], cat[:, 32:33].broadcast_to([COMP_LEN, COMP_DIM]),
                    bb)
                nc.vector.tensor_sub(t2[:], t2[:], t1[:])
                cat2 = sp.tile([COMP_LEN, 2 * COMP_DIM], f32, tag="cat2")
                nc.vector.tensor_add(cat2[:, 0:COMP_DIM], cat[:, 0:COMP_DIM],
                                     t2[:])
                nc.vector.tensor_mul(cat2[:, COMP_DIM:2 * COMP_DIM],
                                     cat2[:, 0:COMP_DIM], cat2[:, 0:COMP_DIM])

                # ---- LN2 global sums + broadcast via PE (sel2 carries 1/2048)
                psd = psdep.tile([COMP_LEN, 1], f32, tag="psDE")
                nc.tensor.matmul(psd[:], cat2[:], ones64[:],
                                 start=True, stop=True)
                sD = sp.tile([COMP_LEN, 1], f32, tag="sD")
                nc.vector.tensor_copy(sD[:], psd[:])
                sDm = sp.tile([COMP_LEN, 2], f32, tag="sDm")
                nc.vector.tensor_mul(
                    sDm[:], sD[:].broadcast_to([COMP_LEN, 2]), sel2[:])
                psf = psdep.tile([COMP_LEN, 2], f32, tag="psDE")
                nc.tensor.matmul(psf[:], ones6464[:], sDm[:],
                                 start=True, stop=True)
                bS = sp.tile([COMP_LEN, 2], f32, tag="bS")
                nc.vector.tensor_copy(bS[:], psf[:])

                mu = bS[:, 0:1]
                mu2 = sp.tile([COMP_LEN, 1], f32, tag="mu2")
                nc.vector.tensor_mul(mu2[:], bS[:, 0:1], bS[:, 0:1])
                ex2 = sp.tile([COMP_LEN, 1], f32, tag="ex2")
                nc.vector.tensor_sub(ex2[:], bS[:, 1:2], mu2[:])
                sd2 = sp.tile([COMP_LEN, 1], f32, tag="sd2")
                nc.scalar.activation(sd2[:], ex2[:],
                                     mybir.ActivationFunctionType.Sqrt,
                                     bias=epsb[0:COMP_LEN, :])
                rr = sp.tile([COMP_LEN, 1], f32, tag="rr")
                nc.vector.reciprocal(rr[:], sd2[:])

                obuf = sp.tile([COMP_LEN, COMP_DIM], f32, tag="obuf")
                nc.vector.tensor_scalar(obuf[:], cat2[:, 0:COMP_DIM],
                                        mu, rr[:],
                                        mybir.AluOpType.subtract,
                                        mybir.AluOpType.mult)
                nc.vector.tensor_mul(obuf[:], obuf[:], g2)
                nc.vector.tensor_add(obuf[:], obuf[:], b2)

                nc.sync.dma_start(out_d.ap()[t], obuf[:])

    nc.compile()
    return nc


def _get_program():
    global _PROGRAM
    if _PROGRAM is None:
        _PROGRAM = _build_program()
    return _PROGRAM


def _prepare_inputs(x, ln1_g, ln1_b, ln2_g, ln2_b, node_idx, stacked_indices):
    """Host-side index preprocessing + weight prep. Returns list of in_maps."""
    node_idx = np.asarray(node_idx).astype(np.int64)
    stacked = np.asarray(stacked_indices).astype(np.int64)
    x = np.asarray(x, dtype=np.float32)
    ln1_g = np.asarray(ln1_g, dtype=np.float32)
    ln1_b = np.asarray(ln1_b, dtype=np.float32)
    ln2_g = np.asarray(ln2_g, dtype=np.float32)
    ln2_b = np.asarray(ln2_b, dtype=np.float32)

    # histogram bt[n, j] = count of node n in basket j  (index preprocessing)
    bt = np.zeros((NUM_NODES, COMP_LEN), dtype=np.float32)
    j_ids = np.broadcast_to(np.arange(COMP_LEN)[:, None], stacked.shape)
    np.add.at(bt, (stacked.ravel(), j_ids.ravel()), 1.0)
    node_used = bt.any(axis=1)

    # weight prep
    wstat = np.zeros((E, NSTAT), dtype=np.float32)
    wstat[np.arange(E), np.arange(E) // 4] = ln1_g
    wstat[:, 32] = 1.0
    wstat_bf = wstat.astype(BF16)
    scv = ln1_g.reshape(COMP_DIM, 4).sum(1)
    bbv = ln1_b.reshape(COMP_DIM, 4).mean(1)
    # sc is used against lambda which already carries 1/max_len (via R4S)
    sc782 = np.broadcast_to(scv, (COMP_LEN, COMP_DIM))
    bb782 = np.broadcast_to(bbv / MAX_LEN, (COMP_LEN, COMP_DIM))
    g2 = ln2_g.reshape(COMP_LEN, COMP_DIM)
    b2 = ln2_b.reshape(COMP_LEN, COMP_DIM)
    cst4 = np.concatenate([sc782, bb782, g2, b2], axis=1).astype(np.float32)

    in_maps = []
    for core in range(N_CORES):
        ts = list(range(core * T_LOC, (core + 1) * T_LOC))
        am = np.zeros((T_LOC, 128, CH, COMP_LEN), dtype=BF16)
        xb = np.empty((T_LOC, NT, E), dtype=BF16)
        for ti, tg in enumerate(ts):
            nt_ids = node_idx[tg, :N_NODE]
            kept = np.flatnonzero(node_used[nt_ids])
            if len(kept) > NT:
                print(f"WARNING: kept token overflow {len(kept)} > {NT}",
                      file=sys.stderr)
                kept = kept[:NT]
            nk = len(kept)
            sel = np.zeros(NT, dtype=np.int64)
            sel[:nk] = kept
            xb[ti] = x[tg, sel, :].astype(BF16)
            a_full = bt[nt_ids[sel], :]
            a_full[nk:, :] = 0.0
            am[ti] = a_full.reshape(CH, 128, COMP_LEN).transpose(1, 0, 2)
        in_maps.append({
            "xb": xb,
            "am": am,
            "wstat": wstat_bf,
            "cst4": cst4,
        })
    return in_maps


def kernel(x, ln1_g, ln1_b, ln2_g, ln2_b, node_idx, stacked_indices,
           n_node=N_NODE, num_nodes=NUM_NODES):
    global LAST_RESULTS
    from concourse.bass_utils import run_bass_kernel_spmd

    nc = _get_program()
    in_maps = _prepare_inputs(x, ln1_g, ln1_b, ln2_g, ln2_b, node_idx,
                              stacked_indices)

    if os.environ.get("KERNEL_SIM"):
        outs = _run_sim(nc, in_maps)
    else:
        res = run_bass_kernel_spmd(
            nc, in_maps, core_ids=list(range(N_CORES)),
            trace=bool(os.environ.get("KERNEL_TRACE")),
        )
        LAST_RESULTS = res
        outs = [r["out"] for r in res.results]

    full = np.concatenate(outs, axis=0)           # [16, 64, 32]
    return full.reshape(T, 1, COMP_LEN * COMP_DIM).astype(np.float32)


def _run_sim(nc, in_maps):
    """CoreSim path (KERNEL_SIM=1): simulate cores serially."""
    from concourse.bass_interp import CoreSim
    outs = []
    ncores = int(os.environ.get("KERNEL_SIM_CORES", "1"))
    for core, im in enumerate(in_maps[:ncores]):
        sim = CoreSim(nc, trace=False)
        for k, v in im.items():
            sim.tensor(k)[:] = v
        sim.simulate(check_with_hw=False)
        outs.append(np.array(sim.tensor("out")))
    for core in range(ncores, len(in_maps)):
        outs.append(np.zeros((T_LOC, COMP_LEN, COMP_DIM), np.float32))
    return outs
